# revision 16
# baseline (speedup 1.0000x reference)
"""Correlation-layer kernel for Trainium2 (8 NeuronCores, data-parallel over batch).

Problem (per batch b):
    corr[k, m] = sum_c x[b, c, u, v] * y[b, c, i, j],  k = v*h+u, m = i*w+j
    out = relu(corr) / sqrt(sum_k relu(corr)^2 + eps)   (normalize over k per m)

Shapes: x, y = (8, 128, 48, 64) fp32 -> out (8, 3072, 48, 64) fp32.
Sharding: 1 batch per core.

Design (v4): m on PARTITIONS, k on free dim ("transposed" vs v3). Per m-tile
(128 m's x 3072 k's):
  - 6 fp16 matmuls (lhsT = y-tile stationary, rhs = x) into 2 psum halves
    [128, 1536]; relu evacuation to fp16 r-tile, halves split ACT/Pool.
  - sum-of-squares over k via ONE DVE tensor_scalar(pow 2) with accum_out
    (4x perf mode, fp16 SBUF) -> [128, 1] fp32.
  - norm: ACT sqrt(ss + eps) then DVE reciprocal ([128,1], tiny).
  - scale: ONE DVE tensor_scalar(mult by per-partition recip) 4x -> o16.
  - output: fp16 [M, K] rows DMA'd contiguously via SP HWDGE (spreads over
    all 16 DMA engines); host transposes to [K, M] and upcasts to fp32.
fp16 output halves HBM write traffic vs fp32 (52us floor instead of 105us).
"""

import sys

sys.path.insert(0, "/opt/trn_rl_repo")

import numpy as np

_BUILD_CACHE = {}

B, C, H, W = 8, 128, 48, 64
K = W * H      # 3072 output channels, k = v*h+u
M = H * W      # 3072 spatial positions, m = i*w+j
MT = M // 128  # 24 m-tiles
HALF = K // 2  # 1536, psum half-tile width (3 psum banks)
EPS = 1e-6

PIPE = 2           # software pipeline depth (stages behind for norm/scale)


def evac_eng(i, h):     # engine for relu evac of half h of m-tile i
    return "dve" if (h == 1 and i % 3 == 0) else "act"


def sq_eng(i):          # engine for the TT square of m-tile i
    return "dve" if i % 12 == 0 else "pool"


def build():
    from concourse import bacc, bass, mybir, tile

    F32 = mybir.dt.float32
    F16 = mybir.dt.float16
    AF = mybir.ActivationFunctionType
    OP = mybir.AluOpType

    nc = bacc.Bacc("TRN2", debug=False, target_bir_lowering=False)

    a_d = nc.dram_tensor("a", [C, K], F16, kind="ExternalInput")   # x, k-major
    y_d = nc.dram_tensor("y", [C, M], F16, kind="ExternalInput")   # y, m-major
    out_d = nc.dram_tensor("out", [M, K], F16, kind="ExternalOutput")
    junk_d = nc.dram_tensor("junkout", [128, 512], F16, kind="ExternalOutput")

    def relu_evac(eng, dst, src):
        if eng == "act":
            nc.scalar.activation(dst, src, AF.Relu)
        elif eng == "pool":
            nc.gpsimd.tensor_scalar_max(dst, src, 0.0)
        else:
            nc.vector.tensor_scalar_max(dst, src, 0.0)

    with tile.TileContext(nc) as tc:
        with (
            tc.tile_pool(name="pers", bufs=1) as pers,
            tc.tile_pool(name="rp", bufs=PIPE + 2) as rp,
            tc.tile_pool(name="sqp", bufs=2) as sqp,
            tc.tile_pool(name="op", bufs=3) as opool,
            tc.tile_pool(name="sm", bufs=PIPE + 2) as sm,
            tc.tile_pool(name="psA", bufs=2, space=bass.MemorySpace.PSUM) as psA,
            tc.tile_pool(name="psJ", bufs=1, space=bass.MemorySpace.PSUM) as psJ,
        ):
            a_t = pers.tile([C, K], F16)
            y_t = pers.tile([C, M], F16)
            eps_t = pers.tile([128, 1], F32)
            nc.sync.dma_start(y_t[:], y_d[:])
            nc.sync.dma_start(a_t[:], a_d[:])
            nc.vector.memset(eps_t[:], EPS)

            rs = {}
            sss = {}
            rcs = {}
            junk_ps = psJ.tile([128, 512], F32, tag="junk")

            def jmm(n=1):
                # dead matmuls that keep the PE active so the HAM clock gate
                # holds the core at 2.4 GHz (idle PE -> half-speed duty)
                for _ in range(n):
                    nc.tensor.matmul(
                        junk_ps[:], y_t[:, 0:128], a_t[:, 0:512],
                        start=True, stop=True, skip_group_check=True,
                    )

            def stage_A(i):
                m0 = i * 128
                r = rp.tile([128, K], F16, tag="r")
                for h in range(2):
                    ps = psA.tile([128, HALF], F32, tag="ps")
                    for j in range(3):
                        k0 = h * HALF + j * 512
                        nc.tensor.matmul(
                            ps[:, j * 512 : (j + 1) * 512],
                            y_t[:, m0 : m0 + 128],
                            a_t[:, k0 : k0 + 512],
                            start=True, stop=True,
                        )
                    relu_evac(evac_eng(i, h),
                              r[:, h * HALF : (h + 1) * HALF], ps[:])
                ss = sm.tile([128, 1], F32, tag="ss")
                sq = sqp.tile([128, K], F16, tag="sq")
                if sq_eng(i) == "pool":
                    nc.gpsimd.tensor_tensor(sq[:], r[:], r[:], OP.mult)
                else:
                    nc.vector.tensor_tensor(sq[:], r[:], r[:], OP.mult)
                sqj = sqp.tile([128, K], F16, tag="sqj")
                nc.vector.tensor_scalar(
                    out=sqj[:], in0=sq[:], scalar1=1.0, scalar2=None,
                    op0=OP.mult, op1=OP.add, accum_out=ss[:],
                )
                rs[i] = r
                sss[i] = ss

            def stage_N(i):
                ss = sss.pop(i)
                srt = sm.tile([128, 1], F32, tag="srt")
                nc.scalar.activation(srt[:], ss[:], AF.Sqrt, bias=eps_t[:])
                rc = sm.tile([128, 1], F32, tag="rc")
                nc.vector.reciprocal(rc[:], srt[:])
                rcs[i] = rc

            def stage_S(i):
                m0 = i * 128
                r = rs.pop(i)
                rc = rcs.pop(i)
                o = opool.tile([128, K], F16, tag="o")
                nc.vector.tensor_scalar(
                    out=o[:], in0=r[:], scalar1=rc[:], scalar2=None,
                    op0=OP.mult,
                )
                nc.sync.dma_start(out_d[m0 : m0 + 128, :], o[:])

            jmm(12)  # warm-up burst after inputs land
            for i in range(MT + PIPE):
                if i < MT:
                    stage_A(i)
                    jmm(8)
                else:
                    jmm(16)
                if 1 <= i < MT + 1:
                    stage_N(i - 1)
                if i >= PIPE:
                    stage_S(i - PIPE)
            junk_sb = sm.tile([128, 512], F16, tag="junksb")
            nc.scalar.activation(junk_sb[:], junk_ps[:], AF.Copy)
            nc.sync.dma_start(junk_d[:], junk_sb[:])

    nc.compile()
    return nc


def get_built():
    if "nc" not in _BUILD_CACHE:
        _BUILD_CACHE["nc"] = build()
    return _BUILD_CACHE["nc"]


def make_in_maps(x, y):
    in_maps = []
    for bi in range(B):
        a = np.ascontiguousarray(
            np.asarray(x)[bi].transpose(0, 2, 1).reshape(C, K)
        ).astype(np.float16)
        ym = np.ascontiguousarray(
            np.asarray(y)[bi].reshape(C, M)
        ).astype(np.float16)
        in_maps.append({"a": a, "y": ym})
    return in_maps


def run(x, y, trace=False):
    from concourse import bass_utils

    nc = get_built()
    in_maps = make_in_maps(x, y)
    res = bass_utils.run_bass_kernel_spmd(
        nc, in_maps, core_ids=list(range(B)), trace=trace
    )
    out = np.stack([
        res.results[bi]["out"].T.astype(np.float32).reshape(K, H, W)
        for bi in range(B)
    ])
    return out, res


def kernel(x, y):
    out, _ = run(x, y, trace=False)
    return out
